# revision 37
# baseline (speedup 1.0000x reference)
"""Trainium2 Bass kernel for nn_AttentionBlock_68624987455817.

Pre-LN causal self-attention block + MLP (B=8, L=1024, E=768, H=12, D=64).

Sharding: data-parallel over batch B=8 across the 8 NeuronCores (one batch
element per core, weights replicated, no collectives). Each core runs the
full block on its [1024, 768] slice.

Per-core dataflow (activations kept feature-major through the matmuls so no
transposes are needed inside attention):
  ph0   LN1 per token tile (bn_stats/bn_aggr), apply on ACT, transpose via
        PE into z1T [E, L]; fully per-tile so ph2 overlaps tile-by-tile.
  ph2   v   = z1 @ wv               (token-major, lhsT = z1T tiles; an extra
        ones column per head makes the P@V matmul emit softmax row-sums)
  ph3   per head pair: qk chunks (q pre-scaled 1/sqrt(D)), then
        S^T = k_h^T q_h -> exp -> P^T (masked); [O^T; sums] = Vaug^T P^T;
        normalize via fast reciprocal + gpsimd partition broadcast.
  ph4   x1 = O @ wproj + x fused with LN2 + transpose -> z2T, per token
        tile; x and x1 stay resident in SBUF (no DRAM round trip).
  ph5   hT = selu(wfc^T @ z2T)      (wfc pre-scaled by selu lambda)
  ph6   out = h @ wout + x1         (token-major, two column passes)

All weights are DMA'd into SBUF once (wqk/wv/wproj at kernel start, wfc
during ph4, wout halves at ph5 start) so no phase waits on weight DMA.

Matmul operand dtype is selectable (KERNEL_MM_DT env): "bf16" (default,
1 cyc/row + fast weight load, rel err ~4e-3) or "f32r" (float32r, rel err
~2e-4). Accumulation is always fp32; LN stats, residuals and the output are
fp32. Softmax skips the max-subtraction (|S| <= ~8 for LN'd inputs so exp
cannot overflow in fp32); causal masking zeroes P^T blocks above the
diagonal.

LN scales fold into the following weight matrices host-side; LN biases and
all linear biases fold into per-feature biases that are only materialized
on-chip when nonzero (all zero for this problem's inputs).
"""
import os
import sys

sys.path.insert(0, "/opt/trn_rl_repo")

import numpy as np
import ml_dtypes

import concourse.bass as bass
from concourse import bacc
import concourse.mybir as mybir
from concourse.tile import TileContext
from concourse import bass_utils
from concourse.masks import make_identity

F32 = mybir.dt.float32
F32R = mybir.dt.float32r
BF16 = mybir.dt.bfloat16
F8E4 = mybir.dt.float8e4
DR = mybir.MatmulPerfMode.DoubleRow
AF = mybir.ActivationFunctionType
OP = mybir.AluOpType
AX = mybir.AxisListType

P = 128
L = 1024
E = 768
H = 12
D = 64
DA = D + 1           # V columns + ones column (row-sum trick)
EC = E // P          # 6 feature chunks
LT = L // P          # 8 token tiles
QC = L // 512        # 2 query chunks
KC2 = 4 * E // P     # 24 chunks of the MLP hidden dim
NCORES = 8

SELU_LAMBDA = 1.0507009873554805
SELU_ALPHA = 1.6732632423543772
SELU_LA = SELU_LAMBDA * SELU_ALPHA
LN_EPS = 1e-6

# fp8 MLP path: wfc/z2/h/wout are fp8e4 and ph5/ph6 run DoubleRow matmuls.
# Weights are pre-scaled by power-of-two factors to sit in fp8e4's normal
# range; the descales fold into existing activation/residual constants.
FC_WSCALE = 16.0     # wfc pre-scale  -> psum = 16*lambda*u
OUT_WSCALE = 32.0    # wout pre-scale
H_SCALE = 16.0       # hT holds 16*selu(u) so the relu term is just max(pt,0)
OUT_DESCALE = 1.0 / (H_SCALE * OUT_WSCALE)

_last_results = None
_build_cache = {}


def _build(gates, mm_dt_name, use_fp8):
    MDT = {"f32r": F32R, "bf16": BF16}[mm_dt_name]
    # transpose-psum staging: how many 128-col chunks fit in one 2KB bank
    TRG = 6 if MDT != F32R else 3
    ZDT = F8E4 if use_fp8 else MDT      # z2T / hT / wfc / wout dtype

    nc = bacc.Bacc("TRN2", target_bir_lowering=False)

    x_d = nc.dram_tensor("x", [L, E], F32, kind="ExternalInput")
    wqk_d = nc.dram_tensor("wqk", [E, 2 * E], MDT, kind="ExternalInput")
    wv_d = nc.dram_tensor("wv", [E, E], MDT, kind="ExternalInput")
    wproj_d = nc.dram_tensor("wproj", [E, E], MDT, kind="ExternalInput")
    wfc_d = nc.dram_tensor("wfc", [E, 4 * E], ZDT, kind="ExternalInput")
    wout_d = nc.dram_tensor("wout", [4 * E, E], ZDT, kind="ExternalInput")
    out_d = nc.dram_tensor("out", [L, E], F32, kind="ExternalOutput")

    bqk_d = bv_d = bproj_d = bfce_d = bfcl_d = bout_d = None
    if gates["bqk"]:
        bqk_d = nc.dram_tensor("bqk", [2 * E], F32, kind="ExternalInput")
    if gates["bv"]:
        bv_d = nc.dram_tensor("bv", [E], F32, kind="ExternalInput")
    if gates["bproj"]:
        bproj_d = nc.dram_tensor("bproj", [E], F32, kind="ExternalInput")
    if gates["bfc"]:
        bfce_d = nc.dram_tensor("bfce", [4 * E], F32, kind="ExternalInput")
        bfcl_d = nc.dram_tensor("bfcl", [4 * E], F32, kind="ExternalInput")
    if gates["bout"]:
        bout_d = nc.dram_tensor("bout", [E], F32, kind="ExternalInput")

    xv = x_d.rearrange("(t p) e -> p t e", p=P)            # [128, 8, 768]
    wqkv = wqk_d.rearrange("(c p) m -> p c m", p=P)        # [128, 6, 1536]
    wvv = wv_d.rearrange("(c p) m -> p c m", p=P)          # [128, 6, 768]
    wprojv = wproj_d.rearrange("(c p) m -> p c m", p=P)    # [128, 6, 768]
    wfcv = wfc_d.rearrange("(c p) m -> p c m", p=P)        # [128, 6, 3072]
    woutv = wout_d.rearrange("(c p) m -> p c m", p=P)      # [128, 24, 768]
    outv = out_d.rearrange("(t p) e -> p t e", p=P)

    with TileContext(nc) as tc:
        with tc.tile_pool(name="persist", bufs=1) as pers:
            # x loads first in program order: everything downstream hangs off
            # them, and the sync engine drains dma_starts in issue order.
            xres = pers.tile([P, LT, E], F32, name="xres")
            for t in range(LT):
                nc.sync.dma_start(xres[:, t, :], xv[:, t, :])

            # mask_tri[p, f] = 1.0 if f >= p else 0.0 (keep where k <= q).
            # Built in f32 (f32r memset/affine_select fail walrus codegen).
            mask_f32 = pers.tile([P, P], F32)
            nc.gpsimd.memset(mask_f32[:], 0.0)
            nc.gpsimd.affine_select(
                out=mask_f32[:], in_=mask_f32[:],
                compare_op=OP.is_ge, fill=1.0, base=-1,
                pattern=[[-1, P]], channel_multiplier=1,
            )
            if MDT == F32R:
                mask_tri = mask_f32[:].bitcast(F32R)
            else:
                mask_b = pers.tile([P, P], BF16)
                nc.vector.tensor_copy(mask_b[:], mask_f32[:])
                mask_tri = mask_b[:]
            ones_f32 = pers.tile([P, LT * H], F32)
            nc.vector.memset(ones_f32[:], 1.0)
            eps_b = pers.tile([P, 1], F32)
            nc.vector.memset(eps_b[:], LN_EPS)
            # warm the ACT spline tables (Sqrt/Identity/Exp) during the x DMA
            # so the first LN tile doesn't eat the ~1.3us-per-set load latency
            warm = pers.tile([P, 1], F32)
            nc.scalar.activation(warm[:], eps_b[:], AF.Sqrt)
            nc.scalar.activation(warm[:], eps_b[:], AF.Identity)
            selu_c = (H_SCALE if use_fp8 else 1.0) * SELU_LA
            lnla_b = pers.tile([P, 1], F32)
            nc.vector.memset(lnla_b[:], float(np.log(selu_c)))

            ident = pers.tile([P, P], F32)
            make_identity(nc, ident)
            ident_m = pers.tile([P, P], MDT)
            nc.vector.tensor_copy(ident_m[:], ident[:])
            if use_fp8:
                ident_z = pers.tile([P, P], F8E4)
                nc.vector.tensor_copy(ident_z[:], ident[:])
            else:
                ident_z = ident_m

            bqk_sb = bv_sb = bproj_sb = bfce_sb = bfcl_sb = bout_sb = None
            if gates["bqk"]:
                bqk_sb = pers.tile([P, 2 * EC], F32)
                nc.sync.dma_start(bqk_sb[:], bqk_d.rearrange("(c p) -> p c", p=P))
            if gates["bv"]:
                bv_sb = pers.tile([P, E], F32)
                nc.sync.dma_start(bv_sb[:], bv_d.to_broadcast((P, E)))
            if gates["bproj"]:
                bproj_sb = pers.tile([P, E], F32)
                nc.sync.dma_start(bproj_sb[:], bproj_d.to_broadcast((P, E)))
            if gates["bfc"]:
                bfce_sb = pers.tile([P, KC2], F32)
                nc.sync.dma_start(bfce_sb[:], bfce_d.rearrange("(c p) -> p c", p=P))
                bfcl_sb = pers.tile([P, KC2], F32)
                nc.sync.dma_start(bfcl_sb[:], bfcl_d.rearrange("(c p) -> p c", p=P))
            if gates["bout"]:
                bout_sb = pers.tile([P, E], F32)
                nc.sync.dma_start(bout_sb[:], bout_d.to_broadcast((P, E)))

            def ln_tile(xsl, zdstT, t, mvp, zp, psp, zdt, identity):
                """LayerNorm one token tile [P, E] and transpose into
                zdstT[:, :, t*P:(t+1)*P]. Fully per-tile: no cross-tile dep."""
                st6 = mvp.tile([P, 2, 6], F32, tag="st6")
                nc.vector.bn_stats(st6[:, 0, :], xsl[:, 0:E // 2])
                nc.vector.bn_stats(st6[:, 1, :], xsl[:, E // 2:E])
                mv = mvp.tile([P, 2], F32, tag="mv")
                nc.vector.bn_aggr(mv[:], st6[:])
                sd = mvp.tile([P, 1], F32, tag="sd")
                nc.scalar.activation(sd[:], mv[:, 1:2], AF.Sqrt, bias=eps_b[:])
                r = mvp.tile([P, 1], F32, tag="r")
                nc.vector.reciprocal(r[:], sd[:])
                mr = mvp.tile([P, 1], F32, tag="mr")
                nc.vector.scalar_tensor_tensor(mr[:], mv[:, 0:1], -1.0, r[:],
                                               OP.mult, OP.mult)
                zt = zp.tile([P, E], zdt, tag="z")
                nc.scalar.activation(zt[:], xsl, AF.Identity,
                                     bias=mr[:], scale=r[:])
                # (XBAR dma_start_transpose here hangs the device when mixed
                # with the concurrent weight-prefetch DMAs; keep PE transposes)
                for g0 in range(0, EC, TRG):
                    gw = min(TRG, EC - g0)
                    pt = psp.tile([P, TRG * P], zdt, tag="tr")
                    for c in range(gw):
                        nc.tensor.transpose(
                            pt[:, c * P:(c + 1) * P],
                            zt[:, (g0 + c) * P:(g0 + c + 1) * P], identity[:])
                    nc.any.tensor_copy(
                        out=zdstT[:, g0:g0 + gw, t * P:(t + 1) * P],
                        in_=pt[:, 0:gw * P].rearrange("p (c q) -> p c q", q=P))

            with tc.tile_pool(name="fm", bufs=1) as fmp:
                z1T = fmp.tile([P, EC, L], MDT, tag="fm")
                with (
                    tc.tile_pool(name="wqkp", bufs=1) as wqkp,
                    tc.tile_pool(name="wvp", bufs=1) as wvp,
                    tc.tile_pool(name="wpp", bufs=1) as wpp,
                    tc.tile_pool(name="otp", bufs=1) as otp,
                    tc.tile_pool(name="vp", bufs=1) as vpool,
                ):
                    wv_sb = wvp.tile([P, EC, E], MDT)
                    nc.sync.dma_start(wv_sb[:], wvv[:])
                    wqk_sb = wqkp.tile([P, EC, 2 * E], MDT)
                    nc.sync.dma_start(wqk_sb[:], wqkv[:])
                    wproj_sb = wpp.tile([P, EC, E], MDT)
                    nc.sync.dma_start(wproj_sb[:], wprojv[:])
                    OT = otp.tile([P, EC, L], MDT)
                    v_aug = vpool.tile([P, LT, H, DA], MDT)
                    nc.vector.tensor_copy(
                        v_aug[:, :, :, D:DA],
                        ones_f32[:].rearrange("p (t h o) -> p t h o", h=H, o=1))

                    # ---------- ph0: LN1 + transpose -> z1T; ph2: v ----------
                    with (
                        tc.tile_pool(name="mv0", bufs=4) as mvp0,
                        tc.tile_pool(name="z0", bufs=4) as zp0,
                        tc.tile_pool(name="ps0", bufs=2, space="PSUM") as ps0,
                        tc.tile_pool(name="ps2", bufs=3, space="PSUM") as ps2,
                    ):
                        for t in range(LT):
                            ln_tile(xres[:, t, :], z1T, t, mvp0, zp0, ps0,
                                    MDT, ident_m)
                        for t in range(LT):
                            for (c0, cw) in ((0, 512), (512, 256)):
                                pt = ps2.tile([P, 512], F32, tag="mm")
                                for kc in range(EC):
                                    nc.tensor.matmul(
                                        pt[:, :cw], z1T[:, kc, t * P:(t + 1) * P],
                                        wv_sb[:, kc, c0:c0 + cw],
                                        start=(kc == 0), stop=(kc == EC - 1),
                                    )
                                # scatter the 64-wide head slices into v_aug
                                h0 = c0 // D
                                nh = cw // D
                                dst = v_aug[:, t, h0:h0 + nh, 0:D]
                                if gates["bv"]:
                                    nc.vector.tensor_tensor(
                                        dst,
                                        pt[:, :cw].rearrange("p (h d) -> p h d", d=D),
                                        bv_sb[:, c0:c0 + cw].rearrange(
                                            "p (h d) -> p h d", d=D),
                                        OP.add)
                                else:
                                    nc.any.tensor_copy(
                                        out=dst,
                                        in_=pt[:, :cw].rearrange(
                                            "p (h d) -> p h d", d=D))

                    # ------- ph3: per head pair qk + attention --------------
                    def qk_compute(c, qk_pair, psqk):
                        # qk matmuls for this pair: oc=c (q), oc=EC+c (k)
                        for i, oc in enumerate((c, EC + c)):
                            psums = [psqk.tile([P, 512], F32, tag="mm",
                                               name=f"qkps{lc}")
                                     for lc in range(QC)]
                            for kc in range(EC):
                                for lc in range(QC):
                                    nc.tensor.matmul(
                                        psums[lc][:],
                                        wqk_sb[:, kc, oc * P:(oc + 1) * P],
                                        z1T[:, kc, lc * 512:(lc + 1) * 512],
                                        start=(kc == 0), stop=(kc == EC - 1),
                                    )
                            for lc in range(QC):
                                dst = qk_pair[:, i, lc * 512:(lc + 1) * 512]
                                if gates["bqk"]:
                                    nc.scalar.activation(
                                        dst, psums[lc][:], AF.Identity,
                                        bias=bqk_sb[:, oc:oc + 1])
                                else:
                                    nc.any.tensor_copy(out=dst, in_=psums[lc][:])

                    def st_exp(qc, kt, qk_pair, PT, ps3s):
                        s0 = kt * P
                        if s0 < 512:
                            segs = [(s0, 512), (512, L)]
                        else:
                            segs = [(s0, L)]
                        # issue both parities' S^T matmuls back-to-back: they
                        # use disjoint PE row groups (lhsT bases 0/64) so the
                        # array runs them concurrently (row packing)
                        psss = []
                        for par in range(2):
                            rows = slice(par * D, par * D + D)
                            pss = ps3s.tile([P, L], F32, tag="st",
                                            name=f"pss{par}")
                            lhs = qk_pair[rows, 1, s0:s0 + P]
                            for (a, b) in segs:
                                nc.tensor.matmul(pss[:, a:b], lhs,
                                                 qk_pair[rows, 0, a:b],
                                                 start=True, stop=True)
                            psss.append(pss)
                        for par in range(2):
                            pt_buf = PT[par]
                            nc.scalar.activation(pt_buf[:, kt, s0:L],
                                                 psss[par][:, s0:L], AF.Exp)
                            nc.vector.tensor_tensor(
                                pt_buf[:, kt, s0:s0 + P],
                                pt_buf[:, kt, s0:s0 + P],
                                mask_tri, OP.mult,
                            )

                    def pv_norm(c, qc, par, PT, ps3v, recp):
                        # P@V for both heads: lhsT = [V_h | 1] so psum row 64
                        # carries the softmax row-sums; the reciprocal
                        # (computed on one row, SBUF — the custom DVE op reads
                        # garbage from PSUM) is partition-broadcast on the
                        # idle GpSimd.
                        q0 = qc * 512
                        h = 2 * c + par
                        pt_buf = PT[par]
                        pso = ps3v.tile([P, 512], F32, tag="pv")
                        kts = list(range(4 * (qc + 1)))
                        for j, kt in enumerate(kts):
                            a = max(kt * P, q0)
                            nc.tensor.matmul(pso[0:DA, a - q0:512],
                                             v_aug[:, kt, h, :],
                                             pt_buf[:, kt, a:q0 + 512],
                                             start=(j == 0),
                                             stop=(j == len(kts) - 1))
                        o_rows = slice(par * D, par * D + D)
                        srow = recp.tile([P, 512], F32, tag="sr")
                        nc.vector.tensor_copy(srow[0:1, :], pso[D:DA, :])
                        rec = recp.tile([P, 512], F32, tag="rc")
                        nc.vector.reciprocal_approx_fast(rec[0:1, :],
                                                         srow[0:1, :])
                        recb = recp.tile([P, 512], F32, tag="rb")
                        nc.gpsimd.partition_broadcast(recb[0:D, :], rec[0:1, :])
                        nc.vector.tensor_tensor(
                            OT[o_rows, c, qc * 512:(qc + 1) * 512],
                            pso[0:D, :], recb[0:D, :], OP.mult,
                        )

                    with (
                        tc.tile_pool(name="qkpp", bufs=2) as qkpp,
                        tc.tile_pool(name="ptp", bufs=1) as ptp,
                        tc.tile_pool(name="recp", bufs=2) as recp,
                        tc.tile_pool(name="psqk", bufs=2, space="PSUM") as psqk,
                        tc.tile_pool(name="ps3s", bufs=2, space="PSUM") as ps3s,
                        tc.tile_pool(name="ps3v", bufs=2, space="PSUM") as ps3v,
                    ):
                        # 4 P^T buffers (2 per parity) so head pair c+1's
                        # S^T/exp can start while pair c's P@V still reads.
                        PT4 = [ptp.tile([P, LT, L], MDT, tag=f"pt{i}",
                                        name=f"pt{i}") for i in range(4)]
                        for c in range(EC):  # head pair (2c, 2c+1)
                            PT = PT4[2 * (c % 2):2 * (c % 2) + 2]
                            qk_pair = qkpp.tile([P, 2, L], MDT, tag="qkpair")
                            qk_compute(c, qk_pair, psqk)
                            for qc in range(QC):
                                for kt in range(4 * qc, 4 * (qc + 1)):
                                    st_exp(qc, kt, qk_pair, PT, ps3s)
                                for par in range(2):
                                    pv_norm(c, qc, par, PT, ps3v, recp)

                    # ------- ph4: proj + residual fused with LN2 ------------
                    # x1res (live thru ph6) and wfc (thru ph5) go on the
                    # right-side SBUF stack so they don't occupy space during
                    # ph0-ph3 (the left stack is near capacity there).
                    x1p = tc.alloc_tile_pool(name="x1res", bufs=1, side="right")
                    wfcp = tc.alloc_tile_pool(name="wfcp", bufs=1, side="right")
                    with (
                        tc.tile_pool(name="mv4", bufs=4) as mvp4,
                        tc.tile_pool(name="z4", bufs=4) as zp4,
                        tc.tile_pool(name="ps4", bufs=3, space="PSUM") as ps4,
                        tc.tile_pool(name="ps45", bufs=2, space="PSUM") as ps45,
                    ):
                        # prefetch wfc during ph4 so ph5 starts immediately
                        wfc_sb = wfcp.tile([P, EC, 4 * E], ZDT)
                        for i in range(4):
                            nc.sync.dma_start(wfc_sb[:, :, i * E:(i + 1) * E],
                                              wfcv[:, :, i * E:(i + 1) * E])
                        x1res = x1p.tile([P, LT, E], F32)
                        z2T = fmp.tile([P, EC, L], ZDT, tag="fm")
                        for t in range(LT):
                            for (c0, cw) in ((0, 512), (512, 256)):
                                pt = ps4.tile([P, 512], F32, tag="mm")
                                for kc in range(EC):
                                    nc.tensor.matmul(
                                        pt[:, :cw], OT[:, kc, t * P:(t + 1) * P],
                                        wproj_sb[:, kc, c0:c0 + cw],
                                        start=(kc == 0), stop=(kc == EC - 1),
                                    )
                                dst = x1res[:, t, c0:c0 + cw]
                                if gates["bproj"]:
                                    nc.vector.tensor_tensor(
                                        dst, pt[:, :cw],
                                        bproj_sb[:, c0:c0 + cw], OP.add)
                                    nc.vector.tensor_tensor(
                                        dst, dst, xres[:, t, c0:c0 + cw], OP.add)
                                else:
                                    nc.vector.tensor_tensor(
                                        dst, pt[:, :cw], xres[:, t, c0:c0 + cw],
                                        OP.add)
                            ln_tile(x1res[:, t, :], z2T, t, mvp4, zp4, ps45,
                                    ZDT, ident_z)

                # ---------------- ph5: fc + selu -> hT ----------------------
                with (
                    tc.tile_pool(name="woa", bufs=1) as woap,
                    tc.tile_pool(name="wob", bufs=1) as wobp,
                    tc.tile_pool(name="htp", bufs=1) as htp,
                ):
                    wo_a = woap.tile([P, KC2, 512], ZDT)
                    nc.sync.dma_start(wo_a[:], woutv[:, :, 0:512])
                    wo_b = wobp.tile([P, KC2, 256], ZDT)
                    nc.sync.dma_start(wo_b[:], woutv[:, :, 512:768])
                    hT = htp.tile([P, KC2, L], ZDT)
                    mm_scale = 1.0 / (FC_WSCALE if use_fp8 else 1.0)
                    with (
                        tc.tile_pool(name="selu", bufs=2) as slp,
                        tc.tile_pool(name="ps5", bufs=3, space="PSUM") as ps5,
                    ):
                        for oc in range(KC2):
                            for lc in range(QC):
                                pt = ps5.tile([P, 512], F32, tag="mm")
                                if use_fp8:
                                    for j in range(EC // 2):
                                        nc.tensor.matmul(
                                            pt[:],
                                            wfc_sb[:, 2 * j:2 * j + 2,
                                                   oc * P:(oc + 1) * P],
                                            z2T[:, 2 * j:2 * j + 2,
                                                lc * 512:(lc + 1) * 512],
                                            start=(j == 0),
                                            stop=(j == EC // 2 - 1),
                                            perf_mode=DR,
                                        )
                                else:
                                    for kc in range(EC):
                                        nc.tensor.matmul(
                                            pt[:],
                                            wfc_sb[:, kc, oc * P:(oc + 1) * P],
                                            z2T[:, kc, lc * 512:(lc + 1) * 512],
                                            start=(kc == 0), stop=(kc == EC - 1),
                                        )
                                pe_t = slp.tile([P, 512], F32, tag="pe")
                                bias = (bfce_sb[:, oc:oc + 1] if gates["bfc"]
                                        else lnla_b[:])
                                nc.scalar.activation(pe_t[:], pt[:], AF.Exp,
                                                     bias=bias,
                                                     scale=mm_scale / SELU_LAMBDA)
                                a_t = slp.tile([P, 512], BF16, tag="at")
                                nc.vector.tensor_scalar(
                                    a_t[:], pe_t[:], selu_c, selu_c,
                                    OP.min, OP.subtract)
                                dst = hT[:, oc, lc * 512:(lc + 1) * 512]
                                if gates["bfc"]:
                                    rl = slp.tile([P, 512], F32, tag="rl")
                                    nc.vector.tensor_scalar(
                                        rl[:], pt[:], bfcl_sb[:, oc:oc + 1],
                                        0.0, OP.add, OP.max)
                                    nc.vector.tensor_tensor(dst, rl[:], a_t[:],
                                                            OP.add)
                                else:
                                    nc.vector.scalar_tensor_tensor(
                                        dst, pt[:], 0.0, a_t[:], OP.max, OP.add)
                    wfcp.release()

                    # ------------ ph6: out = h @ wout + x1 ------------------
                    def out_pass(wo, c0, cw, osp, ps6):
                        for t in range(LT):
                            pt = ps6.tile([P, 512], F32, tag="mm")
                            if use_fp8:
                                for j in range(KC2 // 2):
                                    nc.tensor.matmul(
                                        pt[:, :cw],
                                        hT[:, 2 * j:2 * j + 2, t * P:(t + 1) * P],
                                        wo[:, 2 * j:2 * j + 2, :],
                                        start=(j == 0), stop=(j == KC2 // 2 - 1),
                                        perf_mode=DR,
                                    )
                            else:
                                for kc in range(KC2):
                                    nc.tensor.matmul(
                                        pt[:, :cw], hT[:, kc, t * P:(t + 1) * P],
                                        wo[:, kc, :],
                                        start=(kc == 0), stop=(kc == KC2 - 1),
                                    )
                            ot = osp.tile([P, 512], F32, tag="ot")
                            x1sl = x1res[:, t, c0:c0 + cw]
                            if gates["bout"]:
                                if use_fp8:
                                    nc.vector.scalar_tensor_tensor(
                                        ot[:, :cw], pt[:, :cw], OUT_DESCALE,
                                        bout_sb[:, c0:c0 + cw],
                                        OP.mult, OP.add)
                                else:
                                    nc.vector.tensor_tensor(
                                        ot[:, :cw], pt[:, :cw],
                                        bout_sb[:, c0:c0 + cw], OP.add)
                                nc.vector.tensor_tensor(ot[:, :cw], ot[:, :cw],
                                                        x1sl, OP.add)
                            elif use_fp8:
                                nc.vector.scalar_tensor_tensor(
                                    ot[:, :cw], pt[:, :cw], OUT_DESCALE, x1sl,
                                    OP.mult, OP.add)
                            else:
                                nc.vector.tensor_tensor(ot[:, :cw], pt[:, :cw],
                                                        x1sl, OP.add)
                            nc.sync.dma_start(outv[:, t, c0:c0 + cw],
                                              ot[:, :cw])

                    with (
                        tc.tile_pool(name="osA", bufs=3) as osp,
                        tc.tile_pool(name="ps6A", bufs=6, space="PSUM") as ps6,
                    ):
                        out_pass(wo_a, 0, 512, osp, ps6)
                        out_pass(wo_b, 512, 256, osp, ps6)
                x1p.release()

    nc.finalize()
    return nc


def kernel(**inputs):
    global _last_results

    mm_dt_name = os.environ.get("KERNEL_MM_DT", "bf16")
    # fp8 DoubleRow for the MLP exists behind this flag but is off: both
    # operands in fp8e4 give ~5% relative error per GEMM (the error does not
    # average down with contraction length), which blows the 2e-2 gate.
    use_fp8 = os.environ.get("KERNEL_FP8", "0") == "1" and mm_dt_name == "bf16"

    def arr(name):
        return np.ascontiguousarray(np.asarray(inputs[name], dtype=np.float32))

    x = arr("x")                       # [8, 1024, 768]
    g1 = arr("ln1_scale")
    b1 = arr("ln1_bias")
    w_qkv = arr("w_qkv")               # [768, 2304]
    b_qkv = arr("b_qkv")
    w_proj = arr("w_proj")
    b_proj = arr("b_proj")
    g2 = arr("ln2_scale")
    b2 = arr("ln2_bias")
    w_fc = arr("w_fc")
    b_fc = arr("b_fc")
    w_out = arr("w_out")
    b_out = arr("b_out")

    qscale = np.float32(1.0 / np.sqrt(D))

    w3 = w_qkv.reshape(E, H, 3, D)
    qw = (w3[:, :, 0, :].reshape(E, E) * qscale)
    kw = w3[:, :, 1, :].reshape(E, E)
    vw = w3[:, :, 2, :].reshape(E, E)
    wqk = np.ascontiguousarray(
        np.concatenate([qw, kw], axis=1) * g1[:, None]).astype(np.float32)
    wv = np.ascontiguousarray(vw * g1[:, None]).astype(np.float32)

    bq3 = (b1 @ w_qkv + b_qkv).reshape(H, 3, D)
    bqk = np.concatenate(
        [bq3[:, 0, :].reshape(E) * qscale, bq3[:, 1, :].reshape(E)]).astype(np.float32)
    bv = np.ascontiguousarray(bq3[:, 2, :].reshape(E)).astype(np.float32)

    fc_ws = np.float32(FC_WSCALE if use_fp8 else 1.0)
    selu_c_host = (H_SCALE if use_fp8 else 1.0) * SELU_LA
    wfc_p = np.ascontiguousarray(
        w_fc * g2[:, None] * np.float32(SELU_LAMBDA) * fc_ws).astype(np.float32)
    bfc_eff = (b2 @ w_fc + b_fc).astype(np.float32)
    bfce = (bfc_eff + np.float32(np.log(selu_c_host))).astype(np.float32)
    bfcl = (bfc_eff * np.float32(SELU_LAMBDA) * fc_ws).astype(np.float32)

    gates = {
        "bqk": bool(np.any(bqk != 0)),
        "bv": bool(np.any(bv != 0)),
        "bproj": bool(np.any(b_proj != 0)),
        "bfc": bool(np.any(bfc_eff != 0)),
        "bout": bool(np.any(b_out != 0)),
    }

    key = (tuple(sorted(gates.items())), mm_dt_name, use_fp8)
    if key not in _build_cache:
        _build_cache[key] = _build(gates, mm_dt_name, use_fp8)
    nc = _build_cache[key]

    wdt = np.float32 if mm_dt_name == "f32r" else ml_dtypes.bfloat16
    zdt = ml_dtypes.float8_e4m3 if use_fp8 else wdt
    out_ws = np.float32(OUT_WSCALE if use_fp8 else 1.0)

    def wcast(a):
        return np.ascontiguousarray(a.astype(wdt))

    base = {
        "wqk": wcast(wqk), "wv": wcast(wv),
        "wproj": wcast(w_proj),
        "wfc": np.ascontiguousarray(wfc_p.astype(zdt)),
        "wout": np.ascontiguousarray((w_out * out_ws).astype(zdt)),
    }
    if gates["bqk"]:
        base["bqk"] = bqk
    if gates["bv"]:
        base["bv"] = bv
    if gates["bproj"]:
        base["bproj"] = np.ascontiguousarray(b_proj)
    if gates["bfc"]:
        base["bfce"] = bfce
        base["bfcl"] = bfcl
    if gates["bout"]:
        base["bout"] = np.ascontiguousarray(b_out)

    in_maps = [dict(base, x=np.ascontiguousarray(x[c])) for c in range(NCORES)]
    res = bass_utils.run_bass_kernel_spmd(nc, in_maps, core_ids=list(range(NCORES)))
    _last_results = res
    out = np.stack([res.results[c]["out"] for c in range(NCORES)], axis=0)
    return out.astype(np.float32)


# revision 38
# speedup vs baseline: 1.1973x; 1.1973x over previous
"""Trainium2 Bass kernel for nn_AttentionBlock_68624987455817.

Pre-LN causal self-attention block + MLP (B=8, L=1024, E=768, H=12, D=64).

Sharding: data-parallel over batch B=8 across the 8 NeuronCores (one batch
element per core, weights replicated, no collectives). Each core runs the
full block on its [1024, 768] slice.

Per-core dataflow (activations kept feature-major through the matmuls so no
transposes are needed inside attention):
  ph0   LN1 per token tile (bn_stats/bn_aggr), apply on ACT, transpose via
        PE into z1T [E, L]; fully per-tile so ph2 overlaps tile-by-tile.
  ph2   v   = z1 @ wv               (token-major, lhsT = z1T tiles; an extra
        ones column per head makes the P@V matmul emit softmax row-sums)
  ph3   per head pair: qk chunks (q pre-scaled 1/sqrt(D)), then
        S^T = k_h^T q_h -> exp -> P^T (masked); [O^T; sums] = Vaug^T P^T;
        normalize via fast reciprocal + gpsimd partition broadcast.
  ph4   x1 = O @ wproj + x fused with LN2 + transpose -> z2T, per token
        tile; x and x1 stay resident in SBUF (no DRAM round trip).
  ph5   hT = selu(wfc^T @ z2T)      (wfc pre-scaled by selu lambda)
  ph6   out = h @ wout + x1         (token-major, two column passes)

All weights are DMA'd into SBUF once (wqk/wv/wproj at kernel start, wfc
during ph4, wout halves at ph5 start) so no phase waits on weight DMA.

Matmul operand dtype is selectable (KERNEL_MM_DT env): "bf16" (default,
1 cyc/row + fast weight load, rel err ~4e-3) or "f32r" (float32r, rel err
~2e-4). Accumulation is always fp32; LN stats, residuals and the output are
fp32. Softmax skips the max-subtraction (|S| <= ~8 for LN'd inputs so exp
cannot overflow in fp32); causal masking zeroes P^T blocks above the
diagonal.

LN scales fold into the following weight matrices host-side; LN biases and
all linear biases fold into per-feature biases that are only materialized
on-chip when nonzero (all zero for this problem's inputs).
"""
import os
import sys

sys.path.insert(0, "/opt/trn_rl_repo")

import numpy as np
import ml_dtypes

import concourse.bass as bass
from concourse import bacc
import concourse.mybir as mybir
from concourse.tile import TileContext
from concourse import bass_utils
from concourse.masks import make_identity

F32 = mybir.dt.float32
F32R = mybir.dt.float32r
BF16 = mybir.dt.bfloat16
F8E4 = mybir.dt.float8e4
DR = mybir.MatmulPerfMode.DoubleRow
AF = mybir.ActivationFunctionType
OP = mybir.AluOpType
AX = mybir.AxisListType

P = 128
L = 1024
E = 768
H = 12
D = 64
DA = D + 1           # V columns + ones column (row-sum trick)
EC = E // P          # 6 feature chunks
LT = L // P          # 8 token tiles
QC = L // 512        # 2 query chunks
KC2 = 4 * E // P     # 24 chunks of the MLP hidden dim
NCORES = 8

SELU_LAMBDA = 1.0507009873554805
SELU_ALPHA = 1.6732632423543772
SELU_LA = SELU_LAMBDA * SELU_ALPHA
LN_EPS = 1e-6

# fp8 MLP path: wfc/z2/h/wout are fp8e4 and ph5/ph6 run DoubleRow matmuls.
# Weights are pre-scaled by power-of-two factors to sit in fp8e4's normal
# range; the descales fold into existing activation/residual constants.
FC_WSCALE = 16.0     # wfc pre-scale  -> psum = 16*lambda*u
OUT_WSCALE = 32.0    # wout pre-scale
H_SCALE = 16.0       # hT holds 16*selu(u) so the relu term is just max(pt,0)
OUT_DESCALE = 1.0 / (H_SCALE * OUT_WSCALE)

_last_results = None
_build_cache = {}


def _build(gates, mm_dt_name, use_fp8):
    MDT = {"f32r": F32R, "bf16": BF16}[mm_dt_name]
    # transpose-psum staging: how many 128-col chunks fit in one 2KB bank
    TRG = 6 if MDT != F32R else 3
    ZDT = F8E4 if use_fp8 else MDT      # z2T / hT / wfc / wout dtype

    nc = bacc.Bacc("TRN2", target_bir_lowering=False)

    x_d = nc.dram_tensor("x", [L, E], F32, kind="ExternalInput")
    wqk_d = nc.dram_tensor("wqk", [E, 2 * E], MDT, kind="ExternalInput")
    wv_d = nc.dram_tensor("wv", [E, E], MDT, kind="ExternalInput")
    wproj_d = nc.dram_tensor("wproj", [E, E], MDT, kind="ExternalInput")
    wfc_d = nc.dram_tensor("wfc", [E, 4 * E], ZDT, kind="ExternalInput")
    wout_d = nc.dram_tensor("wout", [4 * E, E], ZDT, kind="ExternalInput")
    out_d = nc.dram_tensor("out", [L, E], F32, kind="ExternalOutput")

    bqk_d = bv_d = bproj_d = bfce_d = bfcl_d = bout_d = None
    if gates["bqk"]:
        bqk_d = nc.dram_tensor("bqk", [2 * E], F32, kind="ExternalInput")
    if gates["bv"]:
        bv_d = nc.dram_tensor("bv", [E], F32, kind="ExternalInput")
    if gates["bproj"]:
        bproj_d = nc.dram_tensor("bproj", [E], F32, kind="ExternalInput")
    if gates["bfc"]:
        bfce_d = nc.dram_tensor("bfce", [4 * E], F32, kind="ExternalInput")
        bfcl_d = nc.dram_tensor("bfcl", [4 * E], F32, kind="ExternalInput")
    if gates["bout"]:
        bout_d = nc.dram_tensor("bout", [E], F32, kind="ExternalInput")

    xv = x_d.rearrange("(t p) e -> p t e", p=P)            # [128, 8, 768]
    wqkv = wqk_d.rearrange("(c p) m -> p c m", p=P)        # [128, 6, 1536]
    wvv = wv_d.rearrange("(c p) m -> p c m", p=P)          # [128, 6, 768]
    wprojv = wproj_d.rearrange("(c p) m -> p c m", p=P)    # [128, 6, 768]
    wfcv = wfc_d.rearrange("(c p) m -> p c m", p=P)        # [128, 6, 3072]
    woutv = wout_d.rearrange("(c p) m -> p c m", p=P)      # [128, 24, 768]
    outv = out_d.rearrange("(t p) e -> p t e", p=P)

    with TileContext(nc) as tc:
        with tc.tile_pool(name="persist", bufs=1) as pers:
            # x loads first in program order: everything downstream hangs off
            # them, and the sync engine drains dma_starts in issue order.
            xres = pers.tile([P, LT, E], F32, name="xres")
            for t in range(LT):
                nc.sync.dma_start(xres[:, t, :], xv[:, t, :])

            # mask_tri[p, f] = 1.0 if f >= p else 0.0 (keep where k <= q).
            # Built in f32 (f32r memset/affine_select fail walrus codegen).
            mask_f32 = pers.tile([P, P], F32)
            nc.gpsimd.memset(mask_f32[:], 0.0)
            nc.gpsimd.affine_select(
                out=mask_f32[:], in_=mask_f32[:],
                compare_op=OP.is_ge, fill=1.0, base=-1,
                pattern=[[-1, P]], channel_multiplier=1,
            )
            if MDT == F32R:
                mask_tri = mask_f32[:].bitcast(F32R)
            else:
                mask_b = pers.tile([P, P], BF16)
                nc.vector.tensor_copy(mask_b[:], mask_f32[:])
                mask_tri = mask_b[:]
            ones_f32 = pers.tile([P, LT * H], F32)
            nc.vector.memset(ones_f32[:], 1.0)
            eps_b = pers.tile([P, 1], F32)
            nc.vector.memset(eps_b[:], LN_EPS)
            # warm the ACT spline tables (Sqrt/Identity/Exp) during the x DMA
            # so the first LN tile doesn't eat the ~1.3us-per-set load latency
            warm = pers.tile([P, 1], F32)
            nc.scalar.activation(warm[:], eps_b[:], AF.Sqrt)
            nc.scalar.activation(warm[:], eps_b[:], AF.Identity)
            selu_c = (H_SCALE if use_fp8 else 1.0) * SELU_LA
            lnla_b = pers.tile([P, 1], F32)
            nc.vector.memset(lnla_b[:], float(np.log(selu_c)))

            ident = pers.tile([P, P], F32)
            make_identity(nc, ident)
            ident_m = pers.tile([P, P], MDT)
            nc.vector.tensor_copy(ident_m[:], ident[:])
            if use_fp8:
                ident_z = pers.tile([P, P], F8E4)
                nc.vector.tensor_copy(ident_z[:], ident[:])
            else:
                ident_z = ident_m

            bqk_sb = bv_sb = bproj_sb = bfce_sb = bfcl_sb = bout_sb = None
            if gates["bqk"]:
                bqk_sb = pers.tile([P, 2 * EC], F32)
                nc.sync.dma_start(bqk_sb[:], bqk_d.rearrange("(c p) -> p c", p=P))
            if gates["bv"]:
                bv_sb = pers.tile([P, E], F32)
                nc.sync.dma_start(bv_sb[:], bv_d.to_broadcast((P, E)))
            if gates["bproj"]:
                bproj_sb = pers.tile([P, E], F32)
                nc.sync.dma_start(bproj_sb[:], bproj_d.to_broadcast((P, E)))
            if gates["bfc"]:
                bfce_sb = pers.tile([P, KC2], F32)
                nc.sync.dma_start(bfce_sb[:], bfce_d.rearrange("(c p) -> p c", p=P))
                bfcl_sb = pers.tile([P, KC2], F32)
                nc.sync.dma_start(bfcl_sb[:], bfcl_d.rearrange("(c p) -> p c", p=P))
            if gates["bout"]:
                bout_sb = pers.tile([P, E], F32)
                nc.sync.dma_start(bout_sb[:], bout_d.to_broadcast((P, E)))

            def ln_tile(xsl, zdstT, t, mvp, zp, psp, zdt, identity):
                """LayerNorm one token tile [P, E] and transpose into
                zdstT[:, :, t*P:(t+1)*P]. Fully per-tile: no cross-tile dep."""
                st6 = mvp.tile([P, 2, 6], F32, tag="st6")
                nc.vector.bn_stats(st6[:, 0, :], xsl[:, 0:E // 2])
                nc.vector.bn_stats(st6[:, 1, :], xsl[:, E // 2:E])
                mv = mvp.tile([P, 2], F32, tag="mv")
                nc.vector.bn_aggr(mv[:], st6[:])
                sd = mvp.tile([P, 1], F32, tag="sd")
                nc.scalar.activation(sd[:], mv[:, 1:2], AF.Sqrt, bias=eps_b[:])
                r = mvp.tile([P, 1], F32, tag="r")
                nc.vector.reciprocal(r[:], sd[:])
                mr = mvp.tile([P, 1], F32, tag="mr")
                nc.vector.scalar_tensor_tensor(mr[:], mv[:, 0:1], -1.0, r[:],
                                               OP.mult, OP.mult)
                zt = zp.tile([P, E], zdt, tag="z")
                nc.scalar.activation(zt[:], xsl, AF.Identity,
                                     bias=mr[:], scale=r[:])
                # (XBAR dma_start_transpose here hangs the device when mixed
                # with the concurrent weight-prefetch DMAs; keep PE transposes)
                for g0 in range(0, EC, TRG):
                    gw = min(TRG, EC - g0)
                    pt = psp.tile([P, TRG * P], zdt, tag="tr")
                    for c in range(gw):
                        nc.tensor.transpose(
                            pt[:, c * P:(c + 1) * P],
                            zt[:, (g0 + c) * P:(g0 + c + 1) * P], identity[:])
                    nc.any.tensor_copy(
                        out=zdstT[:, g0:g0 + gw, t * P:(t + 1) * P],
                        in_=pt[:, 0:gw * P].rearrange("p (c q) -> p c q", q=P))

            with tc.tile_pool(name="fm", bufs=1) as fmp:
                z1T = fmp.tile([P, EC, L], MDT, tag="fm")
                with (
                    tc.tile_pool(name="wqkp", bufs=1) as wqkp,
                    tc.tile_pool(name="wvp", bufs=1) as wvp,
                    tc.tile_pool(name="wpp", bufs=1) as wpp,
                    tc.tile_pool(name="otp", bufs=1) as otp,
                    tc.tile_pool(name="vp", bufs=1) as vpool,
                ):
                    wv_sb = wvp.tile([P, EC, E], MDT)
                    nc.sync.dma_start(wv_sb[:], wvv[:])
                    wqk_sb = wqkp.tile([P, EC, 2 * E], MDT)
                    nc.sync.dma_start(wqk_sb[:], wqkv[:])
                    wproj_sb = wpp.tile([P, EC, E], MDT)
                    nc.sync.dma_start(wproj_sb[:], wprojv[:])
                    OT = otp.tile([P, EC, L], MDT)
                    v_aug = vpool.tile([P, LT, H, DA], MDT)
                    nc.vector.tensor_copy(
                        v_aug[:, :, :, D:DA],
                        ones_f32[:].rearrange("p (t h o) -> p t h o", h=H, o=1))

                    # ---------- ph0: LN1 + transpose -> z1T; ph2: v ----------
                    with (
                        tc.tile_pool(name="mv0", bufs=4) as mvp0,
                        tc.tile_pool(name="z0", bufs=4) as zp0,
                        tc.tile_pool(name="ps0", bufs=2, space="PSUM") as ps0,
                        tc.tile_pool(name="ps2", bufs=3, space="PSUM") as ps2,
                    ):
                        for t in range(LT):
                            ln_tile(xres[:, t, :], z1T, t, mvp0, zp0, ps0,
                                    MDT, ident_m)
                        for t in range(LT):
                            for (c0, cw) in ((0, 512), (512, 256)):
                                pt = ps2.tile([P, 512], F32, tag="mm")
                                for kc in range(EC):
                                    nc.tensor.matmul(
                                        pt[:, :cw], z1T[:, kc, t * P:(t + 1) * P],
                                        wv_sb[:, kc, c0:c0 + cw],
                                        start=(kc == 0), stop=(kc == EC - 1),
                                    )
                                # scatter the 64-wide head slices into v_aug
                                h0 = c0 // D
                                nh = cw // D
                                dst = v_aug[:, t, h0:h0 + nh, 0:D]
                                if gates["bv"]:
                                    nc.vector.tensor_tensor(
                                        dst,
                                        pt[:, :cw].rearrange("p (h d) -> p h d", d=D),
                                        bv_sb[:, c0:c0 + cw].rearrange(
                                            "p (h d) -> p h d", d=D),
                                        OP.add)
                                else:
                                    nc.any.tensor_copy(
                                        out=dst,
                                        in_=pt[:, :cw].rearrange(
                                            "p (h d) -> p h d", d=D))

                    # ------- ph3: per head pair qk + attention --------------
                    def qk_compute(c, qk_pair, psqk):
                        # qk matmuls for this pair: oc=c (q), oc=EC+c (k)
                        for i, oc in enumerate((c, EC + c)):
                            psums = [psqk.tile([P, 512], F32, tag="mm",
                                               name=f"qkps{lc}")
                                     for lc in range(QC)]
                            for kc in range(EC):
                                for lc in range(QC):
                                    nc.tensor.matmul(
                                        psums[lc][:],
                                        wqk_sb[:, kc, oc * P:(oc + 1) * P],
                                        z1T[:, kc, lc * 512:(lc + 1) * 512],
                                        start=(kc == 0), stop=(kc == EC - 1),
                                    )
                            for lc in range(QC):
                                dst = qk_pair[:, i, lc * 512:(lc + 1) * 512]
                                if gates["bqk"]:
                                    nc.scalar.activation(
                                        dst, psums[lc][:], AF.Identity,
                                        bias=bqk_sb[:, oc:oc + 1])
                                else:
                                    nc.any.tensor_copy(out=dst, in_=psums[lc][:])

                    def st_exp(qc, kt, qk_pair, PT, ps3s):
                        s0 = kt * P
                        if s0 < 512:
                            segs = [(s0, 512), (512, L)]
                        else:
                            segs = [(s0, L)]
                        # issue both parities' S^T matmuls back-to-back: they
                        # use disjoint PE row groups (lhsT bases 0/64) so the
                        # array runs them concurrently (row packing)
                        psss = []
                        for par in range(2):
                            rows = slice(par * D, par * D + D)
                            pss = ps3s.tile([P, L], F32, tag="st",
                                            name=f"pss{par}")
                            lhs = qk_pair[rows, 1, s0:s0 + P]
                            for (a, b) in segs:
                                nc.tensor.matmul(pss[:, a:b], lhs,
                                                 qk_pair[rows, 0, a:b],
                                                 start=True, stop=True)
                            psss.append(pss)
                        for par in range(2):
                            pt_buf = PT[par]
                            nc.scalar.activation(pt_buf[:, kt, s0:L],
                                                 psss[par][:, s0:L], AF.Exp)
                            nc.vector.tensor_tensor(
                                pt_buf[:, kt, s0:s0 + P],
                                pt_buf[:, kt, s0:s0 + P],
                                mask_tri, OP.mult,
                            )

                    def pv_norm(c, qc, par, PT, ps3v, recp):
                        # P@V for both heads: lhsT = [V_h | 1] so psum row 64
                        # carries the softmax row-sums; the reciprocal
                        # (computed on one row, SBUF — the custom DVE op reads
                        # garbage from PSUM) is partition-broadcast on the
                        # idle GpSimd.
                        q0 = qc * 512
                        h = 2 * c + par
                        pt_buf = PT[par]
                        pso = ps3v.tile([P, 512], F32, tag="pv")
                        kts = list(range(4 * (qc + 1)))
                        for j, kt in enumerate(kts):
                            a = max(kt * P, q0)
                            nc.tensor.matmul(pso[0:DA, a - q0:512],
                                             v_aug[:, kt, h, :],
                                             pt_buf[:, kt, a:q0 + 512],
                                             start=(j == 0),
                                             stop=(j == len(kts) - 1))
                        o_rows = slice(par * D, par * D + D)
                        srow = recp.tile([P, 512], F32, tag="sr")
                        nc.vector.tensor_copy(srow[0:1, :], pso[D:DA, :])
                        rec = recp.tile([P, 512], F32, tag="rc")
                        nc.vector.reciprocal_approx_fast(rec[0:1, :],
                                                         srow[0:1, :])
                        recb = recp.tile([P, 512], F32, tag="rb")
                        nc.gpsimd.partition_broadcast(recb[0:D, :], rec[0:1, :])
                        nc.vector.tensor_tensor(
                            OT[o_rows, c, qc * 512:(qc + 1) * 512],
                            pso[0:D, :], recb[0:D, :], OP.mult,
                        )

                    with (
                        tc.tile_pool(name="qkpp", bufs=2) as qkpp,
                        tc.tile_pool(name="ptp", bufs=1) as ptp,
                        tc.tile_pool(name="recp", bufs=2) as recp,
                        tc.tile_pool(name="psqk", bufs=2, space="PSUM") as psqk,
                        tc.tile_pool(name="ps3s", bufs=2, space="PSUM") as ps3s,
                        tc.tile_pool(name="ps3v", bufs=2, space="PSUM") as ps3v,
                    ):
                        # 4 P^T buffers (2 per parity) so head pair c+1's
                        # S^T/exp can start while pair c's P@V still reads.
                        PT4 = [ptp.tile([P, LT, L], MDT, tag=f"pt{i}",
                                        name=f"pt{i}") for i in range(4)]
                        for c in range(EC):  # head pair (2c, 2c+1)
                            PT = PT4[2 * (c % 2):2 * (c % 2) + 2]
                            qk_pair = qkpp.tile([P, 2, L], MDT, tag="qkpair")
                            qk_compute(c, qk_pair, psqk)
                            for qc in range(QC):
                                for kt in range(4 * qc, 4 * (qc + 1)):
                                    st_exp(qc, kt, qk_pair, PT, ps3s)
                                for par in range(2):
                                    pv_norm(c, qc, par, PT, ps3v, recp)

                    # ------- ph4: proj + residual fused with LN2 ------------
                    # x1res (live thru ph6) and wfc (thru ph5) go on the
                    # right-side SBUF stack so they don't occupy space during
                    # ph0-ph3 (the left stack is near capacity there).
                    x1p = tc.alloc_tile_pool(name="x1res", bufs=1, side="right")
                    wfcp = tc.alloc_tile_pool(name="wfcp", bufs=1, side="right")
                    with (
                        tc.tile_pool(name="mv4", bufs=4) as mvp4,
                        tc.tile_pool(name="z4", bufs=4) as zp4,
                        tc.tile_pool(name="ps4", bufs=5, space="PSUM") as ps4,
                        tc.tile_pool(name="ps45", bufs=3, space="PSUM") as ps45,
                    ):
                        # prefetch wfc during ph4 so ph5 starts immediately
                        wfc_sb = wfcp.tile([P, EC, 4 * E], ZDT)
                        for i in range(4):
                            nc.sync.dma_start(wfc_sb[:, :, i * E:(i + 1) * E],
                                              wfcv[:, :, i * E:(i + 1) * E])
                        x1res = x1p.tile([P, LT, E], F32)
                        z2T = fmp.tile([P, EC, L], ZDT, tag="fm")
                        for t in range(LT):
                            for (c0, cw) in ((0, 512), (512, 256)):
                                pt = ps4.tile([P, 512], F32, tag="mm")
                                for kc in range(EC):
                                    nc.tensor.matmul(
                                        pt[:, :cw], OT[:, kc, t * P:(t + 1) * P],
                                        wproj_sb[:, kc, c0:c0 + cw],
                                        start=(kc == 0), stop=(kc == EC - 1),
                                    )
                                dst = x1res[:, t, c0:c0 + cw]
                                if gates["bproj"]:
                                    nc.vector.tensor_tensor(
                                        dst, pt[:, :cw],
                                        bproj_sb[:, c0:c0 + cw], OP.add)
                                    nc.vector.tensor_tensor(
                                        dst, dst, xres[:, t, c0:c0 + cw], OP.add)
                                else:
                                    nc.vector.tensor_tensor(
                                        dst, pt[:, :cw], xres[:, t, c0:c0 + cw],
                                        OP.add)
                            ln_tile(x1res[:, t, :], z2T, t, mvp4, zp4, ps45,
                                    ZDT, ident_z)

                # ---------------- ph5: fc + selu -> hT ----------------------
                with (
                    tc.tile_pool(name="woa", bufs=1) as woap,
                    tc.tile_pool(name="wob", bufs=1) as wobp,
                    tc.tile_pool(name="htp", bufs=1) as htp,
                ):
                    wo_a = woap.tile([P, KC2, 512], ZDT)
                    nc.sync.dma_start(wo_a[:], woutv[:, :, 0:512])
                    wo_b = wobp.tile([P, KC2, 256], ZDT)
                    nc.sync.dma_start(wo_b[:], woutv[:, :, 512:768])
                    hT = htp.tile([P, KC2, L], ZDT)
                    mm_scale = 1.0 / (FC_WSCALE if use_fp8 else 1.0)
                    with (
                        tc.tile_pool(name="selu", bufs=2) as slp,
                        tc.tile_pool(name="ps5", bufs=6, space="PSUM") as ps5,
                    ):
                        for oc in range(KC2):
                            for lc in range(QC):
                                pt = ps5.tile([P, 512], F32, tag="mm")
                                if use_fp8:
                                    for j in range(EC // 2):
                                        nc.tensor.matmul(
                                            pt[:],
                                            wfc_sb[:, 2 * j:2 * j + 2,
                                                   oc * P:(oc + 1) * P],
                                            z2T[:, 2 * j:2 * j + 2,
                                                lc * 512:(lc + 1) * 512],
                                            start=(j == 0),
                                            stop=(j == EC // 2 - 1),
                                            perf_mode=DR,
                                        )
                                else:
                                    for kc in range(EC):
                                        nc.tensor.matmul(
                                            pt[:],
                                            wfc_sb[:, kc, oc * P:(oc + 1) * P],
                                            z2T[:, kc, lc * 512:(lc + 1) * 512],
                                            start=(kc == 0), stop=(kc == EC - 1),
                                        )
                                pe_t = slp.tile([P, 512], F32, tag="pe")
                                bias = (bfce_sb[:, oc:oc + 1] if gates["bfc"]
                                        else lnla_b[:])
                                nc.scalar.activation(pe_t[:], pt[:], AF.Exp,
                                                     bias=bias,
                                                     scale=mm_scale / SELU_LAMBDA)
                                a_t = slp.tile([P, 512], BF16, tag="at")
                                nc.vector.tensor_scalar(
                                    a_t[:], pe_t[:], selu_c, selu_c,
                                    OP.min, OP.subtract)
                                dst = hT[:, oc, lc * 512:(lc + 1) * 512]
                                if gates["bfc"]:
                                    rl = slp.tile([P, 512], F32, tag="rl")
                                    nc.vector.tensor_scalar(
                                        rl[:], pt[:], bfcl_sb[:, oc:oc + 1],
                                        0.0, OP.add, OP.max)
                                    nc.vector.tensor_tensor(dst, rl[:], a_t[:],
                                                            OP.add)
                                else:
                                    nc.vector.scalar_tensor_tensor(
                                        dst, pt[:], 0.0, a_t[:], OP.max, OP.add)
                    wfcp.release()

                    # ------------ ph6: out = h @ wout + x1 ------------------
                    def out_pass(wo, c0, cw, osp, ps6):
                        for t in range(LT):
                            pt = ps6.tile([P, 512], F32, tag="mm")
                            if use_fp8:
                                for j in range(KC2 // 2):
                                    nc.tensor.matmul(
                                        pt[:, :cw],
                                        hT[:, 2 * j:2 * j + 2, t * P:(t + 1) * P],
                                        wo[:, 2 * j:2 * j + 2, :],
                                        start=(j == 0), stop=(j == KC2 // 2 - 1),
                                        perf_mode=DR,
                                    )
                            else:
                                for kc in range(KC2):
                                    nc.tensor.matmul(
                                        pt[:, :cw], hT[:, kc, t * P:(t + 1) * P],
                                        wo[:, kc, :],
                                        start=(kc == 0), stop=(kc == KC2 - 1),
                                    )
                            ot = osp.tile([P, 512], F32, tag="ot")
                            x1sl = x1res[:, t, c0:c0 + cw]
                            if gates["bout"]:
                                if use_fp8:
                                    nc.vector.scalar_tensor_tensor(
                                        ot[:, :cw], pt[:, :cw], OUT_DESCALE,
                                        bout_sb[:, c0:c0 + cw],
                                        OP.mult, OP.add)
                                else:
                                    nc.vector.tensor_tensor(
                                        ot[:, :cw], pt[:, :cw],
                                        bout_sb[:, c0:c0 + cw], OP.add)
                                nc.vector.tensor_tensor(ot[:, :cw], ot[:, :cw],
                                                        x1sl, OP.add)
                            elif use_fp8:
                                nc.vector.scalar_tensor_tensor(
                                    ot[:, :cw], pt[:, :cw], OUT_DESCALE, x1sl,
                                    OP.mult, OP.add)
                            else:
                                nc.vector.tensor_tensor(ot[:, :cw], pt[:, :cw],
                                                        x1sl, OP.add)
                            nc.sync.dma_start(outv[:, t, c0:c0 + cw],
                                              ot[:, :cw])

                    with (
                        tc.tile_pool(name="osA", bufs=3) as osp,
                        tc.tile_pool(name="ps6A", bufs=6, space="PSUM") as ps6,
                    ):
                        out_pass(wo_a, 0, 512, osp, ps6)
                        out_pass(wo_b, 512, 256, osp, ps6)
                x1p.release()

    nc.finalize()
    return nc


def kernel(**inputs):
    global _last_results

    mm_dt_name = os.environ.get("KERNEL_MM_DT", "bf16")
    # fp8 DoubleRow for the MLP exists behind this flag but is off: both
    # operands in fp8e4 give ~5% relative error per GEMM (the error does not
    # average down with contraction length), which blows the 2e-2 gate.
    use_fp8 = os.environ.get("KERNEL_FP8", "0") == "1" and mm_dt_name == "bf16"

    def arr(name):
        return np.ascontiguousarray(np.asarray(inputs[name], dtype=np.float32))

    x = arr("x")                       # [8, 1024, 768]
    g1 = arr("ln1_scale")
    b1 = arr("ln1_bias")
    w_qkv = arr("w_qkv")               # [768, 2304]
    b_qkv = arr("b_qkv")
    w_proj = arr("w_proj")
    b_proj = arr("b_proj")
    g2 = arr("ln2_scale")
    b2 = arr("ln2_bias")
    w_fc = arr("w_fc")
    b_fc = arr("b_fc")
    w_out = arr("w_out")
    b_out = arr("b_out")

    qscale = np.float32(1.0 / np.sqrt(D))

    w3 = w_qkv.reshape(E, H, 3, D)
    qw = (w3[:, :, 0, :].reshape(E, E) * qscale)
    kw = w3[:, :, 1, :].reshape(E, E)
    vw = w3[:, :, 2, :].reshape(E, E)
    wqk = np.ascontiguousarray(
        np.concatenate([qw, kw], axis=1) * g1[:, None]).astype(np.float32)
    wv = np.ascontiguousarray(vw * g1[:, None]).astype(np.float32)

    bq3 = (b1 @ w_qkv + b_qkv).reshape(H, 3, D)
    bqk = np.concatenate(
        [bq3[:, 0, :].reshape(E) * qscale, bq3[:, 1, :].reshape(E)]).astype(np.float32)
    bv = np.ascontiguousarray(bq3[:, 2, :].reshape(E)).astype(np.float32)

    fc_ws = np.float32(FC_WSCALE if use_fp8 else 1.0)
    selu_c_host = (H_SCALE if use_fp8 else 1.0) * SELU_LA
    wfc_p = np.ascontiguousarray(
        w_fc * g2[:, None] * np.float32(SELU_LAMBDA) * fc_ws).astype(np.float32)
    bfc_eff = (b2 @ w_fc + b_fc).astype(np.float32)
    bfce = (bfc_eff + np.float32(np.log(selu_c_host))).astype(np.float32)
    bfcl = (bfc_eff * np.float32(SELU_LAMBDA) * fc_ws).astype(np.float32)

    gates = {
        "bqk": bool(np.any(bqk != 0)),
        "bv": bool(np.any(bv != 0)),
        "bproj": bool(np.any(b_proj != 0)),
        "bfc": bool(np.any(bfc_eff != 0)),
        "bout": bool(np.any(b_out != 0)),
    }

    key = (tuple(sorted(gates.items())), mm_dt_name, use_fp8)
    if key not in _build_cache:
        _build_cache[key] = _build(gates, mm_dt_name, use_fp8)
    nc = _build_cache[key]

    wdt = np.float32 if mm_dt_name == "f32r" else ml_dtypes.bfloat16
    zdt = ml_dtypes.float8_e4m3 if use_fp8 else wdt
    out_ws = np.float32(OUT_WSCALE if use_fp8 else 1.0)

    def wcast(a):
        return np.ascontiguousarray(a.astype(wdt))

    base = {
        "wqk": wcast(wqk), "wv": wcast(wv),
        "wproj": wcast(w_proj),
        "wfc": np.ascontiguousarray(wfc_p.astype(zdt)),
        "wout": np.ascontiguousarray((w_out * out_ws).astype(zdt)),
    }
    if gates["bqk"]:
        base["bqk"] = bqk
    if gates["bv"]:
        base["bv"] = bv
    if gates["bproj"]:
        base["bproj"] = np.ascontiguousarray(b_proj)
    if gates["bfc"]:
        base["bfce"] = bfce
        base["bfcl"] = bfcl
    if gates["bout"]:
        base["bout"] = np.ascontiguousarray(b_out)

    in_maps = [dict(base, x=np.ascontiguousarray(x[c])) for c in range(NCORES)]
    res = bass_utils.run_bass_kernel_spmd(nc, in_maps, core_ids=list(range(NCORES)))
    _last_results = res
    out = np.stack([res.results[c]["out"] for c in range(NCORES)], axis=0)
    return out.astype(np.float32)
